# revision 17
# baseline (speedup 1.0000x reference)
"""Trainium2 Bass kernel for attention-energies softmax.

Reference computation:
    proj     = enc @ W.T + b          # [S, H]
    energies = proj @ hidden          # [S]
    attn     = softmax(energies)      # [1, 1, S]

Algebraic rewrite used here (identical math, ~1000x less compute):
    energies = enc @ (W.T @ hidden) + (b . hidden)
The scalar (b . hidden) shifts every energy equally, so softmax is
unchanged; we drop it. That turns the [S,H]x[H,H] matmul into a
[S,H]x[H] matvec, making the problem HBM-bound on reading enc (128MB).

Distribution across 8 NeuronCores:
  - enc sharded along S (4096 rows/core).
  - v = W.T @ hidden: each core computes its 128-column slice of v from
    a host-sliced W[:, i*128:(i+1)*128] (8 PE matmuls), then
    AllGather -> full v, broadcast across partitions into PSUM via a
    ones-matmul.
  - Each core: local energies via fused DVE multiply+reduce
    (scalar_tensor_tensor with accum_out, second operand read from
    PSUM), local softmax stats (max via gpsimd partition_all_reduce,
    sum via a ones-matmul), AllGather of the 2 stats, global
    renormalization, writes its 4096-long slice of attn.

DMA strategy: enc is streamed in 16 x 1MB chunks, alternating between
the two HWDGE rings (sync / scalar) so both drain concurrently; small
control DMAs ride SWDGE (gpsimd) to stay out of the ring FIFOs.

Shapes are hardcoded: H=1024, S=32768, 8 cores.
"""

import sys

import numpy as np

for _p in ("/opt/trn_rl_repo", "/root/.axon_site/_ro/trn_rl_repo"):
    try:
        import concourse  # noqa: F401

        break
    except ImportError:
        if _p not in sys.path:
            sys.path.insert(0, _p)

H = 1024
S = 32768
NCORES = 8
P = 128               # SBUF partitions
S_LOC = S // NCORES   # 4096 rows per core
T = S_LOC // P        # 32 energy columns per partition
G = 8                 # DMA groups for enc (2MB each, alternating rings)
U = T // G            # tiles per DMA group

_CACHE = {}


def _build_program(G=G, U=U, reps=1):
    import concourse.bacc as bacc
    import concourse.bass_isa as bass_isa
    import concourse.mybir as mybir
    import concourse.tile as tile

    fp32 = mybir.dt.float32
    Alu = mybir.AluOpType
    Act = mybir.ActivationFunctionType
    Axis = mybir.AxisListType

    T = G * U
    S_LOC = P * T

    nc = bacc.Bacc("TRN2", num_devices=NCORES)

    enc = nc.declare_dram_parameter("enc", [S_LOC, H], fp32, isOutput=False)
    wsl = nc.declare_dram_parameter("wsl", [H, P], fp32, isOutput=False)
    hid = nc.declare_dram_parameter("hid", [H], fp32, isOutput=False)
    attn = nc.declare_dram_parameter("attn", [S_LOC], fp32, isOutput=True)

    cc_v_in = nc.dram_tensor("cc_v_in", [P], fp32)
    cc_v_out = nc.dram_tensor("cc_v_out", [H], fp32, addr_space="Shared")
    cc_s_in = nc.dram_tensor("cc_s_in", [2], fp32)
    cc_s_out = nc.dram_tensor("cc_s_out", [2 * NCORES], fp32, addr_space="Shared")

    groups = [list(range(NCORES))]

    def body(cpool, epool, pspool):
        # ---- constants ----
        ones_row = cpool.tile([1, P], fp32, tag="ones_row")   # lhsT for bcast
        nc.vector.memset(ones_row[:], 1.0)
        ones_col = cpool.tile([P, 1], fp32, tag="ones_col")   # lhsT for psum
        nc.vector.memset(ones_col[:], 1.0)

        # ---- v = W.T @ hidden (this core's 128-column slice) ----
        # On the sync ring: the scalar ring opens with LoadActFuncSet which
        # would delay these latency-critical loads.
        # hid_sb[p, k] = hidden[k*128 + p]
        hid_sb = cpool.tile([P, 8], fp32, tag="hid_sb")
        nc.sync.dma_start(hid_sb[:], hid[:].rearrange("(k p) -> p k", p=P))
        # w_sb[p, k, h] = wsl[k*128 + p, h]
        w_sb = cpool.tile([P, 8, P], fp32, tag="w_sb")
        nc.sync.dma_start(w_sb[:], wsl[:].rearrange("(k p) h -> p k h", p=P))

        v_ps = pspool.tile([P, 1], fp32, tag="v_ps")
        for k in range(8):
            nc.tensor.matmul(
                v_ps[:],
                lhsT=w_sb[:, k, :],
                rhs=hid_sb[:, k : k + 1],
                start=(k == 0),
                stop=(k == 7),
            )
        v_loc = cpool.tile([P, 1], fp32, tag="v_loc")
        nc.vector.tensor_copy(v_loc[:], v_ps[:])
        nc.gpsimd.dma_start(
            cc_v_in[:].rearrange("(p one) -> p one", one=1), v_loc[:]
        )
        nc.gpsimd.collective_compute(
            "AllGather",
            Alu.bypass,
            replica_groups=groups,
            ins=[cc_v_in[:]],
            outs=[cc_v_out[:]],
        )
        # broadcast-read v into all 128 partitions with one stride-0 DMA
        v_bc = cpool.tile([P, H], fp32, tag="v_bc")
        nc.gpsimd.dma_start(
            v_bc[:],
            cc_v_out[:].rearrange("(one h) -> one h", one=1).broadcast_to([P, H]),
        )

        # ---- energies: e[p, t] = enc_row(p*T + t) . v ----
        e = cpool.tile([P, T], fp32, tag="e")
        prod = cpool.tile([P, H], fp32, tag="prod")  # discarded product
        enc_r = enc[:].rearrange("(p g u) h -> g p u h", p=P, g=G, u=U)
        for g in range(G):
            eg = epool.tile([P, U, H], fp32, tag="eg")
            dma_eng = nc.scalar if (g % 2 == 0) else nc.sync
            dma_eng.dma_start(eg[:], enc_r[g])
            for u in range(U):
                t = g * U + u
                # out = (in0 * 1.0) * in1 ; accum_out = sum(out)
                nc.vector.scalar_tensor_tensor(
                    out=prod[:],
                    in0=eg[:, u, :],
                    scalar=1.0,
                    in1=v_bc[:],
                    op0=Alu.mult,
                    op1=Alu.mult,
                    accum_out=e[:, t : t + 1],
                )

        # ---- local softmax stats ----
        mx = cpool.tile([P, 1], fp32, tag="mx")
        nc.vector.tensor_reduce(mx[:], e[:], axis=Axis.X, op=Alu.max)
        m_b = cpool.tile([P, 1], fp32, tag="m_b")
        nc.gpsimd.partition_all_reduce(
            m_b[:], mx[:], channels=P, reduce_op=bass_isa.ReduceOp.max
        )
        negm = cpool.tile([P, 1], fp32, tag="negm")
        nc.vector.tensor_scalar_mul(negm[:], m_b[:], -1.0)
        p_exp = cpool.tile([P, T], fp32, tag="p_exp")
        srow = cpool.tile([P, 1], fp32, tag="srow")
        nc.scalar.activation(
            p_exp[:], e[:], Act.Exp, bias=negm[:], scale=1.0, accum_out=srow[:]
        )
        # sum srow across partitions on the PE: ones[128,1].T @ srow[128,1]
        s_ps = pspool.tile([1, 1], fp32, tag="s_ps")
        nc.tensor.matmul(s_ps[:], lhsT=ones_col[:], rhs=srow[:], start=True, stop=True)

        # ---- exchange (max, sumexp) with the other cores ----
        st2 = cpool.tile([1, 2], fp32, tag="st2")
        nc.vector.tensor_copy(st2[:, 0:1], m_b[0:1, :])
        nc.vector.tensor_copy(st2[:, 1:2], s_ps[:])
        # the HWDGE rings are drained by now; lower first-byte than SWDGE
        nc.scalar.dma_start(cc_s_in[:].rearrange("(one x) -> one x", one=1), st2[:])
        nc.gpsimd.collective_compute(
            "AllGather",
            Alu.bypass,
            replica_groups=groups,
            ins=[cc_s_in[:]],
            outs=[cc_s_out[:]],
        )
        stats = cpool.tile([1, 2 * NCORES], fp32, tag="stats")
        nc.sync.dma_start(
            stats[:], cc_s_out[:].rearrange("(one x) -> one x", one=1)
        )
        stats_r = stats[:].rearrange("a (i two) -> a i two", two=2)
        m_view = stats_r[:, :, 0]  # [1, 8]
        s_view = stats_r[:, :, 1]  # [1, 8]

        # ---- global max / normalizer ----
        Mg = cpool.tile([1, 1], fp32, tag="Mg")
        nc.vector.tensor_reduce(Mg[:], m_view, axis=Axis.X, op=Alu.max)
        negM = cpool.tile([1, 1], fp32, tag="negM")
        nc.vector.tensor_scalar_mul(negM[:], Mg[:], -1.0)
        ti = cpool.tile([1, NCORES], fp32, tag="ti")
        nc.scalar.activation(ti[:], m_view, Act.Exp, bias=negM[:], scale=1.0)
        tz = cpool.tile([1, NCORES], fp32, tag="tz")
        Z = cpool.tile([1, 1], fp32, tag="Z")
        nc.vector.scalar_tensor_tensor(
            out=tz[:],
            in0=ti[:],
            scalar=1.0,
            in1=s_view,
            op0=Alu.mult,
            op1=Alu.mult,
            accum_out=Z[:],
        )
        Zr = cpool.tile([1, 1], fp32, tag="Zr")
        nc.vector.reciprocal(Zr[:], Z[:])
        r0 = cpool.tile([1, 1], fp32, tag="r0")
        nc.scalar.activation(r0[:], m_b[0:1, :], Act.Exp, bias=negM[:], scale=1.0)
        a0 = cpool.tile([1, 1], fp32, tag="a0")
        nc.vector.tensor_mul(a0[:], r0[:], Zr[:])
        # broadcast alpha across partitions on the PE into PSUM
        alpha = pspool.tile([P, 1], fp32, tag="alpha")
        nc.tensor.matmul(alpha[:], lhsT=ones_row[:], rhs=a0[:], start=True, stop=True)

        # ---- attn slice = p_exp * alpha ----
        outp = cpool.tile([P, T], fp32, tag="outp")
        nc.vector.tensor_scalar_mul(outp[:], p_exp[:], alpha[:])
        nc.sync.dma_start(attn[:].rearrange("(p t) -> p t", p=P), outp[:])

    with tile.TileContext(nc) as tc:
        with (
            tc.tile_pool(name="const", bufs=1) as cpool,
            tc.tile_pool(name="encp", bufs=min(G, 8)) as epool,
            tc.tile_pool(name="psum", bufs=1, space="PSUM") as pspool,
        ):
            for _rep in range(reps):
                body(cpool, epool, pspool)

    nc.compile()
    return nc


def _get_program():
    if "nc" not in _CACHE:
        _CACHE["nc"] = _build_program()
    return _CACHE["nc"]


def make_in_maps(hidden, encoder_outputs, W):
    hidden = np.ascontiguousarray(np.asarray(hidden, dtype=np.float32))
    enc = np.ascontiguousarray(np.asarray(encoder_outputs, dtype=np.float32))
    W = np.asarray(W, dtype=np.float32)
    in_maps = []
    for i in range(NCORES):
        in_maps.append(
            {
                "enc": np.ascontiguousarray(enc[i * S_LOC : (i + 1) * S_LOC]),
                "wsl": np.ascontiguousarray(W[:, i * P : (i + 1) * P]),
                "hid": hidden,
            }
        )
    return in_maps


def kernel(hidden, encoder_outputs, W, b, **_unused):
    from concourse.bass_utils import run_bass_kernel_spmd

    nc = _get_program()
    in_maps = make_in_maps(hidden, encoder_outputs, W)
    res = run_bass_kernel_spmd(nc, in_maps, core_ids=list(range(NCORES)))
    out = np.concatenate([res.results[i]["attn"] for i in range(NCORES)])
    return out.reshape(1, 1, S).astype(np.float32)


# revision 18
# speedup vs baseline: 1.0772x; 1.0772x over previous
"""Trainium2 Bass kernel for attention-energies softmax.

Reference computation:
    proj     = enc @ W.T + b          # [S, H]
    energies = proj @ hidden          # [S]
    attn     = softmax(energies)      # [1, 1, S]

Algebraic rewrite used here (identical math, ~1000x less compute):
    energies = enc @ (W.T @ hidden) + (b . hidden)
The scalar (b . hidden) shifts every energy equally, so softmax is
unchanged; we drop it. That turns the [S,H]x[H,H] matmul into a
[S,H]x[H] matvec, making the problem HBM-bound on reading enc (128MB).

Distribution across 8 NeuronCores:
  - enc sharded along S (4096 rows/core).
  - v = W.T @ hidden: each core computes its 128-column slice of v from
    a host-sliced W[:, i*128:(i+1)*128] (8 PE matmuls), then
    AllGather -> full v, then one stride-0 DMA broadcasts v into all
    128 SBUF partitions.
  - Each core: local energies via fused DVE multiply+reduce
    (scalar_tensor_tensor with accum_out), local softmax stats (max via
    gpsimd partition_all_reduce, sum via a ones-matmul on the PE),
    AllGather of the 2 stats, global renormalization, writes its
    4096-long slice of attn.

DMA strategy: enc is streamed in 8 x 2MB chunks, alternating between
the two HWDGE rings (scalar / sync) so both drain concurrently;
mid-kernel control DMAs ride SWDGE (gpsimd) to stay out of the ring
FIFOs while enc is streaming.

Shapes are hardcoded: H=1024, S=32768, 8 cores.
"""

import sys

import numpy as np

for _p in ("/opt/trn_rl_repo", "/root/.axon_site/_ro/trn_rl_repo"):
    try:
        import concourse  # noqa: F401

        break
    except ImportError:
        if _p not in sys.path:
            sys.path.insert(0, _p)

H = 1024
S = 32768
NCORES = 8
P = 128               # SBUF partitions
S_LOC = S // NCORES   # 4096 rows per core
T = S_LOC // P        # 32 energy columns per partition
G = 8                 # DMA groups for enc (2MB each, alternating rings)
U = T // G            # tiles per DMA group

_CACHE = {}


def _build_program(G=G, U=U, reps=1):
    import concourse.bacc as bacc
    import concourse.bass_isa as bass_isa
    import concourse.mybir as mybir
    import concourse.tile as tile

    fp32 = mybir.dt.float32
    Alu = mybir.AluOpType
    Act = mybir.ActivationFunctionType
    Axis = mybir.AxisListType

    T = G * U
    S_LOC = P * T

    nc = bacc.Bacc("TRN2", num_devices=NCORES)

    enc = nc.declare_dram_parameter("enc", [S_LOC, H], fp32, isOutput=False)
    wsl = nc.declare_dram_parameter("wsl", [H, P], fp32, isOutput=False)
    hid = nc.declare_dram_parameter("hid", [H], fp32, isOutput=False)
    attn = nc.declare_dram_parameter("attn", [S_LOC], fp32, isOutput=True)

    cc_v_in = nc.dram_tensor("cc_v_in", [P], fp32)
    cc_v_out = nc.dram_tensor("cc_v_out", [H], fp32, addr_space="Shared")
    cc_s_in = nc.dram_tensor("cc_s_in", [2], fp32)
    cc_s_out = nc.dram_tensor("cc_s_out", [2 * NCORES], fp32, addr_space="Shared")

    groups = [list(range(NCORES))]

    def body(cpool, epool, pspool):
        # ---- constants ----
        ones_row = cpool.tile([1, P], fp32, tag="ones_row")   # lhsT for bcast
        nc.vector.memset(ones_row[:], 1.0)
        ones_col = cpool.tile([P, 1], fp32, tag="ones_col")   # lhsT for psum
        nc.vector.memset(ones_col[:], 1.0)

        # ---- v = W.T @ hidden (this core's 128-column slice) ----
        # On the sync ring: the scalar ring opens with LoadActFuncSet which
        # would delay these latency-critical loads.
        # hid_sb[p, k] = hidden[k*128 + p]
        hid_sb = cpool.tile([P, 8], fp32, tag="hid_sb")
        nc.sync.dma_start(hid_sb[:], hid[:].rearrange("(k p) -> p k", p=P))
        # w_sb[p, k, h] = wsl[k*128 + p, h]
        w_sb = cpool.tile([P, 8, P], fp32, tag="w_sb")
        nc.sync.dma_start(w_sb[:], wsl[:].rearrange("(k p) h -> p k h", p=P))

        v_ps = pspool.tile([P, 1], fp32, tag="v_ps")
        for k in range(8):
            nc.tensor.matmul(
                v_ps[:],
                lhsT=w_sb[:, k, :],
                rhs=hid_sb[:, k : k + 1],
                start=(k == 0),
                stop=(k == 7),
            )
        v_loc = cpool.tile([P, 1], fp32, tag="v_loc")
        nc.vector.tensor_copy(v_loc[:], v_ps[:])
        nc.gpsimd.dma_start(
            cc_v_in[:].rearrange("(p one) -> p one", one=1), v_loc[:]
        )
        nc.gpsimd.collective_compute(
            "AllGather",
            Alu.bypass,
            replica_groups=groups,
            ins=[cc_v_in[:]],
            outs=[cc_v_out[:]],
        )
        # broadcast-read v into all 128 partitions with one stride-0 DMA
        v_bc = cpool.tile([P, H], fp32, tag="v_bc")
        nc.gpsimd.dma_start(
            v_bc[:],
            cc_v_out[:].rearrange("(one h) -> one h", one=1).broadcast_to([P, H]),
        )

        # ---- energies: e[p, t] = enc_row(p*T + t) . v ----
        e = cpool.tile([P, T], fp32, tag="e")
        prod = cpool.tile([P, H], fp32, tag="prod")  # discarded product
        enc_r = enc[:].rearrange("(p g u) h -> g p u h", p=P, g=G, u=U)
        for g in range(G):
            eg = epool.tile([P, U, H], fp32, tag="eg")
            dma_eng = nc.scalar if (g % 2 == 0) else nc.sync
            dma_eng.dma_start(eg[:], enc_r[g])
            for u in range(U):
                t = g * U + u
                # out = (in0 * 1.0) * in1 ; accum_out = sum(out)
                nc.vector.scalar_tensor_tensor(
                    out=prod[:],
                    in0=eg[:, u, :],
                    scalar=1.0,
                    in1=v_bc[:],
                    op0=Alu.mult,
                    op1=Alu.mult,
                    accum_out=e[:, t : t + 1],
                )

        # ---- local softmax stats ----
        mx = cpool.tile([P, 1], fp32, tag="mx")
        nc.vector.tensor_reduce(mx[:], e[:], axis=Axis.X, op=Alu.max)
        m_b = cpool.tile([P, 1], fp32, tag="m_b")
        nc.gpsimd.partition_all_reduce(
            m_b[:], mx[:], channels=P, reduce_op=bass_isa.ReduceOp.max
        )
        negm = cpool.tile([P, 1], fp32, tag="negm")
        nc.vector.tensor_scalar_mul(negm[:], m_b[:], -1.0)
        p_exp = cpool.tile([P, T], fp32, tag="p_exp")
        srow = cpool.tile([P, 1], fp32, tag="srow")
        nc.scalar.activation(
            p_exp[:], e[:], Act.Exp, bias=negm[:], scale=1.0, accum_out=srow[:]
        )
        # sum srow across partitions on the PE: ones[128,1].T @ srow[128,1]
        s_ps = pspool.tile([1, 1], fp32, tag="s_ps")
        nc.tensor.matmul(s_ps[:], lhsT=ones_col[:], rhs=srow[:], start=True, stop=True)

        # ---- exchange (max, sumexp) with the other cores ----
        st2 = cpool.tile([1, 2], fp32, tag="st2")
        nc.vector.tensor_copy(st2[:, 0:1], m_b[0:1, :])
        nc.vector.tensor_copy(st2[:, 1:2], s_ps[:])
        # the HWDGE rings are drained by now; lower first-byte than SWDGE
        nc.scalar.dma_start(cc_s_in[:].rearrange("(one x) -> one x", one=1), st2[:])
        nc.gpsimd.collective_compute(
            "AllGather",
            Alu.bypass,
            replica_groups=groups,
            ins=[cc_s_in[:]],
            outs=[cc_s_out[:]],
        )
        stats = cpool.tile([1, 2 * NCORES], fp32, tag="stats")
        nc.sync.dma_start(
            stats[:], cc_s_out[:].rearrange("(one x) -> one x", one=1)
        )
        stats_r = stats[:].rearrange("a (i two) -> a i two", two=2)
        m_view = stats_r[:, :, 0]  # [1, 8]
        s_view = stats_r[:, :, 1]  # [1, 8]

        # ---- global max / normalizer ----
        Mg = cpool.tile([1, 1], fp32, tag="Mg")
        nc.vector.tensor_reduce(Mg[:], m_view, axis=Axis.X, op=Alu.max)
        negM = cpool.tile([1, 1], fp32, tag="negM")
        nc.vector.tensor_scalar_mul(negM[:], Mg[:], -1.0)
        ti = cpool.tile([1, NCORES], fp32, tag="ti")
        nc.scalar.activation(ti[:], m_view, Act.Exp, bias=negM[:], scale=1.0)
        tz = cpool.tile([1, NCORES], fp32, tag="tz")
        Z = cpool.tile([1, 1], fp32, tag="Z")
        nc.vector.scalar_tensor_tensor(
            out=tz[:],
            in0=ti[:],
            scalar=1.0,
            in1=s_view,
            op0=Alu.mult,
            op1=Alu.mult,
            accum_out=Z[:],
        )
        Zr = cpool.tile([1, 1], fp32, tag="Zr")
        nc.vector.reciprocal(Zr[:], Z[:])
        r0 = cpool.tile([1, 1], fp32, tag="r0")
        nc.scalar.activation(r0[:], m_b[0:1, :], Act.Exp, bias=negM[:], scale=1.0)
        a0 = cpool.tile([1, 1], fp32, tag="a0")
        nc.vector.tensor_mul(a0[:], r0[:], Zr[:])
        # broadcast alpha across partitions on the PE into PSUM
        alpha = pspool.tile([P, 1], fp32, tag="alpha")
        nc.tensor.matmul(alpha[:], lhsT=ones_row[:], rhs=a0[:], start=True, stop=True)

        # ---- attn slice = p_exp * alpha ----
        outp = cpool.tile([P, T], fp32, tag="outp")
        nc.vector.tensor_scalar_mul(outp[:], p_exp[:], alpha[:])
        nc.sync.dma_start(attn[:].rearrange("(p t) -> p t", p=P), outp[:])

    with tile.TileContext(nc) as tc:
        with (
            tc.tile_pool(name="const", bufs=1) as cpool,
            tc.tile_pool(name="encp", bufs=min(G, 8)) as epool,
            tc.tile_pool(name="psum", bufs=1, space="PSUM") as pspool,
        ):
            for _rep in range(reps):
                body(cpool, epool, pspool)

    nc.compile()
    return nc


def _get_program():
    if "nc" not in _CACHE:
        _CACHE["nc"] = _build_program()
    return _CACHE["nc"]


def make_in_maps(hidden, encoder_outputs, W):
    hidden = np.ascontiguousarray(np.asarray(hidden, dtype=np.float32))
    enc = np.ascontiguousarray(np.asarray(encoder_outputs, dtype=np.float32))
    W = np.asarray(W, dtype=np.float32)
    in_maps = []
    for i in range(NCORES):
        in_maps.append(
            {
                "enc": np.ascontiguousarray(enc[i * S_LOC : (i + 1) * S_LOC]),
                "wsl": np.ascontiguousarray(W[:, i * P : (i + 1) * P]),
                "hid": hidden,
            }
        )
    return in_maps


def kernel(hidden, encoder_outputs, W, b, **_unused):
    from concourse.bass_utils import run_bass_kernel_spmd

    nc = _get_program()
    in_maps = make_in_maps(hidden, encoder_outputs, W)
    res = run_bass_kernel_spmd(nc, in_maps, core_ids=list(range(NCORES)))
    out = np.concatenate([res.results[i]["attn"] for i in range(NCORES)])
    return out.reshape(1, 1, S).astype(np.float32)


# revision 27
# speedup vs baseline: 1.1090x; 1.0295x over previous
"""Trainium2 Bass kernel for attention-energies softmax.

Reference computation:
    proj     = enc @ W.T + b          # [S, H]
    energies = proj @ hidden          # [S]
    attn     = softmax(energies)      # [1, 1, S]

Algebraic rewrite used here (identical math, ~1000x less compute):
    energies = enc @ (W.T @ hidden) + (b . hidden)
The scalar (b . hidden) shifts every energy equally, so softmax is
unchanged; we drop it. That turns the [S,H]x[H,H] matmul into a
[S,H]x[H] matvec, making the problem HBM-bound on reading enc (128MB).

Distribution across 8 NeuronCores:
  - enc sharded along S (4096 rows/core).
  - v = W.T @ hidden: each core computes its 128-column slice of v from
    a host-sliced W[:, i*128:(i+1)*128] (8 PE matmuls), then
    AllGather -> full v, then one stride-0 DMA broadcasts v into all
    128 SBUF partitions.
  - Each core: local energies via fused DVE multiply+reduce
    (scalar_tensor_tensor with accum_out), local softmax stats (max via
    gpsimd partition_all_reduce, sum via a ones-matmul on the PE),
    AllGather of the 2 stats, global renormalization, writes its
    4096-long slice of attn.

DMA strategy: enc is streamed in 8 x 2MB chunks, alternating between
the two HWDGE rings (scalar / sync) so both drain concurrently;
mid-kernel control DMAs ride SWDGE (gpsimd) to stay out of the ring
FIFOs while enc is streaming.

Shapes are hardcoded: H=1024, S=32768, 8 cores.
"""

import sys

import numpy as np

for _p in ("/opt/trn_rl_repo", "/root/.axon_site/_ro/trn_rl_repo"):
    try:
        import concourse  # noqa: F401

        break
    except ImportError:
        if _p not in sys.path:
            sys.path.insert(0, _p)

H = 1024
S = 32768
NCORES = 8
P = 128               # SBUF partitions
S_LOC = S // NCORES   # 4096 rows per core
T = S_LOC // P        # 32 energy columns per partition
G = 8                 # DMA groups for enc (2MB each, alternating rings)
U = T // G            # tiles per DMA group

_CACHE = {}


def _build_program(G=G, U=U, reps=1, mode="full"):
    # mode: "full" | "noag2" (local normalize, no stats AllGather)
    #       | "nopar" (also skip cross-partition max; per-partition exp bias)
    #       | "dma" (enc DMAs + cheap consumer only — measures the DMA floor)
    import concourse.bacc as bacc
    import concourse.bass_isa as bass_isa
    import concourse.mybir as mybir
    import concourse.tile as tile

    fp32 = mybir.dt.float32
    Alu = mybir.AluOpType
    Act = mybir.ActivationFunctionType
    Axis = mybir.AxisListType

    T = G * U
    S_LOC = P * T

    nc = bacc.Bacc("TRN2", num_devices=NCORES)

    enc = nc.declare_dram_parameter("enc", [S_LOC, H], fp32, isOutput=False)
    wsl = nc.declare_dram_parameter("wsl", [H, P], fp32, isOutput=False)
    hid = nc.declare_dram_parameter("hid", [H], fp32, isOutput=False)
    attn = nc.declare_dram_parameter("attn", [S_LOC], fp32, isOutput=True)

    cc_v_in = nc.dram_tensor("cc_v_in", [P], fp32)
    cc_v_out = nc.dram_tensor("cc_v_out", [H], fp32, addr_space="Shared")
    cc_s_in = nc.dram_tensor("cc_s_in", [2], fp32)
    cc_s_out = nc.dram_tensor("cc_s_out", [2 * NCORES], fp32, addr_space="Shared")

    groups = [list(range(NCORES))]

    def body(cpool, epool, pspool):
        if mode == "dma":
            acc = cpool.tile([P, 1], fp32, tag="acc")
            enc_r0 = enc[:].rearrange("(p g u) h -> g p u h", p=P, g=G, u=U)
            for g in range(G):
                eg = epool.tile([P, U, H], fp32, tag="eg")
                dma_eng = nc.scalar if (g % 2 == 0) else nc.sync
                dma_eng.dma_start(eg[:], enc_r0[g])
                nc.vector.tensor_reduce(
                    acc[:], eg[:, 0, 0:128], axis=Axis.X, op=Alu.max
                )
            outp = cpool.tile([P, T], fp32, tag="outp")
            nc.vector.memset(outp[:], 0.0)
            nc.vector.tensor_copy(outp[:, 0:1], acc[:])
            nc.sync.dma_start(attn[:].rearrange("(p t) -> p t", p=P), outp[:])
            return

        # ---- constants ----
        ones_row = cpool.tile([1, P], fp32, tag="ones_row")   # lhsT for bcast
        nc.vector.memset(ones_row[:], 1.0)
        ones_col = cpool.tile([P, 1], fp32, tag="ones_col")   # lhsT for psum
        nc.vector.memset(ones_col[:], 1.0)
        ident = cpool.tile([P, P], fp32, tag="ident")         # for PE transpose
        nc.gpsimd.memset(ident[:], 0.0)
        nc.gpsimd.affine_select(
            out=ident[:],
            in_=ident[:],
            compare_op=mybir.AluOpType.not_equal,
            fill=1.0,
            base=0,
            pattern=[[-1, P]],
            channel_multiplier=1,
        )

        # ---- v = W.T @ hidden (this core's 128-column slice) ----
        # On the sync ring: the scalar ring opens with LoadActFuncSet which
        # would delay these latency-critical loads.
        # hid_sb[p, k] = hidden[k*128 + p]
        hid_sb = cpool.tile([P, 8], fp32, tag="hid_sb")
        nc.sync.dma_start(hid_sb[:], hid[:].rearrange("(k p) -> p k", p=P))
        # w_sb[p, k, h] = wsl[k*128 + p, h]
        w_sb = cpool.tile([P, 8, P], fp32, tag="w_sb")
        nc.sync.dma_start(w_sb[:], wsl[:].rearrange("(k p) h -> p k h", p=P))

        v_ps = pspool.tile([P, 1], fp32, tag="v_ps")
        for k in range(8):
            nc.tensor.matmul(
                v_ps[:],
                lhsT=w_sb[:, k, :],
                rhs=hid_sb[:, k : k + 1],
                start=(k == 0),
                stop=(k == 7),
            )
        v_loc = cpool.tile([P, 1], fp32, tag="v_loc")
        nc.vector.tensor_copy(v_loc[:], v_ps[:])
        nc.gpsimd.dma_start(
            cc_v_in[:].rearrange("(p one) -> p one", one=1), v_loc[:]
        )
        nc.gpsimd.collective_compute(
            "AllGather",
            Alu.bypass,
            replica_groups=groups,
            ins=[cc_v_in[:]],
            outs=[cc_v_out[:]],
        )
        # broadcast-read v into all 128 partitions with one stride-0 DMA
        v_bc = cpool.tile([P, H], fp32, tag="v_bc")
        nc.gpsimd.dma_start(
            v_bc[:],
            cc_v_out[:].rearrange("(one h) -> one h", one=1).broadcast_to([P, H]),
        )

        # ---- energies: e[p, t] = enc_row(p*T + t) . v ----
        e = cpool.tile([P, T], fp32, tag="e")
        prod = cpool.tile([P, H], fp32, tag="prod")  # discarded product
        enc_r = enc[:].rearrange("(p g u) h -> g p u h", p=P, g=G, u=U)
        for g in range(G):
            eg = epool.tile([P, U, H], fp32, tag="eg")
            dma_eng = nc.scalar if (g % 2 == 0) else nc.sync
            dma_eng.dma_start(eg[:], enc_r[g])
            for u in range(U):
                t = g * U + u
                # out = (in0 * 1.0) * in1 ; accum_out = sum(out)
                nc.vector.scalar_tensor_tensor(
                    out=prod[:],
                    in0=eg[:, u, :],
                    scalar=1.0,
                    in1=v_bc[:],
                    op0=Alu.mult,
                    op1=Alu.mult,
                    accum_out=e[:, t : t + 1],
                )

        # ---- local softmax stats ----
        mx = cpool.tile([P, 1], fp32, tag="mx")
        nc.vector.tensor_reduce(mx[:], e[:], axis=Axis.X, op=Alu.max)
        M0 = cpool.tile([1, 1], fp32, tag="M0")
        if mode == "nopar":
            m_b = mx[:]
        else:
            # cross-partition max: PE transpose [128,1]->[1,128], DVE reduce,
            # then PE ones-matmul broadcast back to [128,1] (in PSUM).
            # (gpsimd partition_all_reduce measures much slower here.)
            mxT = pspool.tile([1, P], fp32, tag="mxT")
            nc.tensor.transpose(mxT[:], mx[:], ident[:])
            nc.vector.tensor_reduce(M0[:], mxT[:], axis=Axis.X, op=Alu.max)
            m_ps = pspool.tile([P, 1], fp32, tag="m_ps")
            nc.tensor.matmul(
                m_ps[:], lhsT=ones_row[:], rhs=M0[:], start=True, stop=True
            )
            m_b = m_ps[:]
        negm = cpool.tile([P, 1], fp32, tag="negm")
        nc.vector.tensor_scalar_mul(negm[:], m_b, -1.0)
        p_exp = cpool.tile([P, T], fp32, tag="p_exp")
        srow = cpool.tile([P, 1], fp32, tag="srow")
        nc.scalar.activation(
            p_exp[:], e[:], Act.Exp, bias=negm[:], scale=1.0, accum_out=srow[:]
        )
        if mode in ("noag2", "nopar"):
            sinv = cpool.tile([P, 1], fp32, tag="sinv")
            nc.vector.reciprocal(sinv[:], srow[:])
            outp = cpool.tile([P, T], fp32, tag="outp")
            nc.vector.tensor_scalar_mul(outp[:], p_exp[:], sinv[:])
            nc.sync.dma_start(attn[:].rearrange("(p t) -> p t", p=P), outp[:])
            return
        # sum srow across partitions on the PE: ones[128,1].T @ srow[128,1]
        s_ps = pspool.tile([1, 1], fp32, tag="s_ps")
        nc.tensor.matmul(s_ps[:], lhsT=ones_col[:], rhs=srow[:], start=True, stop=True)

        # ---- exchange (max, sumexp) with the other cores ----
        st2 = cpool.tile([1, 2], fp32, tag="st2")
        nc.vector.tensor_copy(st2[:, 0:1], M0[:])
        nc.vector.tensor_copy(st2[:, 1:2], s_ps[:])
        # the HWDGE rings are drained by now; lower first-byte than SWDGE
        nc.scalar.dma_start(cc_s_in[:].rearrange("(one x) -> one x", one=1), st2[:])
        nc.gpsimd.collective_compute(
            "AllGather",
            Alu.bypass,
            replica_groups=groups,
            ins=[cc_s_in[:]],
            outs=[cc_s_out[:]],
        )
        stats = cpool.tile([1, 2 * NCORES], fp32, tag="stats")
        nc.sync.dma_start(
            stats[:], cc_s_out[:].rearrange("(one x) -> one x", one=1)
        )
        stats_r = stats[:].rearrange("a (i two) -> a i two", two=2)
        m_view = stats_r[:, :, 0]  # [1, 8]
        s_view = stats_r[:, :, 1]  # [1, 8]

        # ---- global max / normalizer ----
        Mg = cpool.tile([1, 1], fp32, tag="Mg")
        nc.vector.tensor_reduce(Mg[:], m_view, axis=Axis.X, op=Alu.max)
        negM = cpool.tile([1, 1], fp32, tag="negM")
        nc.vector.tensor_scalar_mul(negM[:], Mg[:], -1.0)
        ti = cpool.tile([1, NCORES], fp32, tag="ti")
        nc.scalar.activation(ti[:], m_view, Act.Exp, bias=negM[:], scale=1.0)
        tz = cpool.tile([1, NCORES], fp32, tag="tz")
        Z = cpool.tile([1, 1], fp32, tag="Z")
        nc.vector.scalar_tensor_tensor(
            out=tz[:],
            in0=ti[:],
            scalar=1.0,
            in1=s_view,
            op0=Alu.mult,
            op1=Alu.mult,
            accum_out=Z[:],
        )
        Zr = cpool.tile([1, 1], fp32, tag="Zr")
        nc.vector.reciprocal(Zr[:], Z[:])
        r0 = cpool.tile([1, 1], fp32, tag="r0")
        nc.scalar.activation(r0[:], M0[:], Act.Exp, bias=negM[:], scale=1.0)
        a0 = cpool.tile([1, 1], fp32, tag="a0")
        nc.vector.tensor_mul(a0[:], r0[:], Zr[:])
        # broadcast alpha across partitions on the PE into PSUM
        alpha = pspool.tile([P, 1], fp32, tag="alpha")
        nc.tensor.matmul(alpha[:], lhsT=ones_row[:], rhs=a0[:], start=True, stop=True)

        # ---- attn slice = p_exp * alpha ----
        outp = cpool.tile([P, T], fp32, tag="outp")
        nc.vector.tensor_scalar_mul(outp[:], p_exp[:], alpha[:])
        nc.sync.dma_start(attn[:].rearrange("(p t) -> p t", p=P), outp[:])

    with tile.TileContext(nc) as tc:
        with (
            tc.tile_pool(name="const", bufs=1) as cpool,
            tc.tile_pool(name="encp", bufs=min(G, 8)) as epool,
            tc.tile_pool(name="psum", bufs=1, space="PSUM") as pspool,
        ):
            for _rep in range(reps):
                body(cpool, epool, pspool)

    nc.compile()
    return nc


def _get_program():
    if "nc" not in _CACHE:
        _CACHE["nc"] = _build_program()
    return _CACHE["nc"]


def make_in_maps(hidden, encoder_outputs, W):
    hidden = np.ascontiguousarray(np.asarray(hidden, dtype=np.float32))
    enc = np.ascontiguousarray(np.asarray(encoder_outputs, dtype=np.float32))
    W = np.asarray(W, dtype=np.float32)
    in_maps = []
    for i in range(NCORES):
        in_maps.append(
            {
                "enc": np.ascontiguousarray(enc[i * S_LOC : (i + 1) * S_LOC]),
                "wsl": np.ascontiguousarray(W[:, i * P : (i + 1) * P]),
                "hid": hidden,
            }
        )
    return in_maps


def kernel(hidden, encoder_outputs, W, b, **_unused):
    from concourse.bass_utils import run_bass_kernel_spmd

    nc = _get_program()
    in_maps = make_in_maps(hidden, encoder_outputs, W)
    res = run_bass_kernel_spmd(nc, in_maps, core_ids=list(range(NCORES)))
    out = np.concatenate([res.results[i]["attn"] for i in range(NCORES)])
    return out.reshape(1, 1, S).astype(np.float32)
